# revision 2
# baseline (speedup 1.0000x reference)
"""Trainium2 Bass kernel for the BMoIE (dense mixture-of-experts) network, v2.

Network (per sample):
    alpha = softmax(x @ gate_w + gate_b)                       # [B, 8]
    h = relu(sum_k alpha_k * (h @ w_l[k] + b_l[k]))            # 3 hidden blocks
    out = sum_k alpha_k * (h @ wo[k] + bo[k])                  # output block

Strategy: data-parallel over 8 NeuronCores (2048 rows each, SPMD, no
collectives). Activations are feature-major ("hT", [128 feat x batch])
as the stationary matmul operand; expert weights stream as the moving
operand (512-col bf16, one expert per matmul -- a matmul output cannot
cross a 2KB PSUM bank, so 512 fp32 cols is the hard cap). With the PE
transposes gone (below), all 8 PSUM banks hold the 8 per-expert z
tiles, so no GEMM ever waits on the combine chain freeing a bank.

All transposes ride the DMA X-bar instead of the PE:
  - x is converted to bf16 on the host and DMA-transposed HBM->SBUF
    straight into feature-major tiles (no PE transpose, no ACT copy).
  - Between layers, the combined activation (bf16) is DMA-transposed
    SBUF->SBUF back to feature-major (relu happens before, on ScalarE).
This removes every PE transpose (~14 us/stack) plus the identity
matrix build (GpSimd launch latency) from the critical path, leaving
the PE stream = pure GEMM at its 437 us/stack roofline.

The alpha-weighted expert sum: experts 1,3 are scaled on ScalarE and
pre-summed on VectorE into gs; the VectorE STT chain then runs
gs -> +a0*z0 -> +a2*z2 -> ... -> +a7*z7 with the LAST link writing
bf16 directly, so only one DVE op trails the final GEMM (the old
standalone final-add is folded into the chain seed). The relu and the
SBUF->SBUF transpose for tile t are deferred to tile t+1's combine
slot on ScalarE, so the relu (which waits on the DVE chain tail)
never head-of-line-blocks the next tile's z reads in ACT's in-order
queue, and the transpose-DMA dispatch (ACT is a HWDGE engine) finds
its input already materialized. Weight DMAs ride the SP queue,
untouched by per-tile dependencies.
"""

import sys

sys.path.insert(0, "/opt/trn_rl_repo")

from contextlib import ExitStack

import numpy as np

import concourse.bass as bass
import concourse.mybir as mybir
import concourse.tile as tile
from concourse import bacc
from concourse.bass_utils import run_bass_kernel_spmd

P = 128           # partitions
D = 512           # model dim (= hidden dim)
K = 8             # experts
NPAIR = K // 2
NCORES = 8
B = 16384
R = B // NCORES   # rows per core
NT = R // P       # 16 batch tiles per core
NCH = D // P      # 4 feature chunks
F32 = mybir.dt.float32
FR = mybir.dt.float32r
BF16 = mybir.dt.bfloat16
AF = mybir.ActivationFunctionType
ALU = mybir.AluOpType

W_NAMES = ("w0", "w1", "w2", "wo")
B_NAMES = ("b0", "b1", "b2", "bo")


def _build(has_gate_b, has_bias, w_bufs=20, ht_bufs=20, repeat=1, mode="full"):
    """Trace + compile the per-core kernel. has_bias is a 4-tuple of bools.

    repeat>1 runs the whole 4-layer stack that many times (same weights,
    full DMA traffic each time) — used only for timing measurements.
    mode: "full" | "pe_only" (GEMMs only, wrong results) for ablation.
    """
    MD = BF16
    any_bias = any(has_bias)

    nc = bacc.Bacc("TRN2", target_bir_lowering=False, num_devices=NCORES)
    x = nc.dram_tensor("x", [R, D], MD, kind="ExternalInput")
    gate_w = nc.dram_tensor("gate_w", [D, K], MD, kind="ExternalInput")
    gate_b = nc.dram_tensor("gate_b", [K], F32, kind="ExternalInput")
    ws = [nc.dram_tensor(n, [K, D, D], MD, kind="ExternalInput") for n in W_NAMES]
    bs = [nc.dram_tensor(n, [K, D], FR, kind="ExternalInput") for n in B_NAMES]
    ident = (nc.dram_tensor("ident", [P, P], F32, kind="ExternalInput")
             if any_bias else None)
    out = nc.dram_tensor("out", [R, D], F32, kind="ExternalOutput")

    with tile.TileContext(nc) as tc, ExitStack() as ctx:
        cst = ctx.enter_context(tc.tile_pool(name="cst", bufs=1))
        wpool = ctx.enter_context(tc.tile_pool(name="wpool", bufs=w_bufs))
        htp = ctx.enter_context(tc.tile_pool(name="htp", bufs=ht_bufs))
        xhtp = ctx.enter_context(tc.tile_pool(name="xhtp", bufs=NT // 4))
        accp = ctx.enter_context(tc.tile_pool(name="accp", bufs=3))
        accbp = ctx.enter_context(tc.tile_pool(name="accbp", bufs=12))
        smp = ctx.enter_context(tc.tile_pool(name="smp", bufs=4))
        zp = ctx.enter_context(tc.tile_pool(name="zp", bufs=8, space="PSUM"))

        # gate_w [512, 8] -> [128, 4, 8]: chunk c holds rows c*128+p
        gw = cst.tile([P, NCH, K], MD, tag="gw")
        nc.sync.dma_start(gw[:], gate_w.rearrange("(c p) k -> p c k", p=P))

        ident_sb = None
        if any_bias:
            ident_sb = cst.tile([P, P], F32, tag="ident")
            nc.sync.dma_start(ident_sb[:], ident[:, :])

        gb_bc = None
        if has_gate_b:
            ones_row = cst.tile([1, P], F32, tag="ones_row")
            nc.vector.memset(ones_row[:], 1.0)
            gb_row = cst.tile([1, K], F32, tag="gb_row")
            nc.sync.dma_start(gb_row[:], gate_b[None, :])
            gb_ps = zp.tile([P, D], F32, tag="z")
            nc.tensor.matmul(gb_ps[:, :K], ones_row[:], gb_row[:])
            gb_bc = cst.tile([P, K], F32, tag="gb_bc")
            nc.scalar.activation(gb_bc[:], gb_ps[:, :K], AF.Copy)

        bl_sb = [None] * 4
        alphaT = None
        if any_bias:
            for li in range(4):
                if has_bias[li]:
                    blt = cst.tile([K, D], FR, tag=f"bl{li}")
                    nc.sync.dma_start(blt[:], bs[li][:, :])
                    bl_sb[li] = blt
            alphaT = cst.tile([K, R], FR, tag="alphaT")

        alpha = cst.tile([P, NT * K], F32, tag="alpha")

        # ---- prologue: DMA-transpose x to feature-major (4 tiles per
        # X-bar transfer: fewer HWDGE holds), then gate ----
        hT = {}

        def x_transpose(q):
            xht = xhtp.tile([P, NCH, 4 * P], MD, tag="xht")
            nc.sync.dma_start_transpose(xht[:], x[q * 4 * P:(q + 1) * 4 * P, :])
            for tt in range(4):
                hT[(0, q * 4 + tt)] = xht[:, :, tt * P:(tt + 1) * P]

        # x transposes first (layer-0 GEMMs consume hT tile-by-tile);
        # layer-0 weights follow; later layers' weight DMAs are spread
        # one-per-tile across the previous layer (below)
        for q in range(NT // 4):
            x_transpose(q)
        wsrs = [w.rearrange("k (c p) o -> k p c o", p=P) for w in ws]
        wt = []
        for k in range(K):
            w_t = wpool.tile([P, NCH, D], MD, tag="w", name=f"w_0_{k}")
            nc.scalar.dma_start(w_t[:], wsrs[0][k])
            wt.append(w_t)
        for t in range(NT):
            ht = hT[(0, t)]

            lg = zp.tile([P, D], F32, tag="z")
            for c in range(NCH):
                nc.tensor.matmul(
                    lg[:, :K],
                    ht[:, c, :P],
                    gw[:, c, :],
                    start=(c == 0),
                    stop=(c == NCH - 1),
                )
            ex = smp.tile([P, K], F32, tag="ex")
            if has_gate_b:
                nc.vector.scalar_tensor_tensor(
                    ex[:], lg[:, :K], 1.0, gb_bc[:], op0=ALU.mult, op1=ALU.add
                )
                nc.scalar.activation(ex[:], ex[:], AF.Exp)
            else:
                nc.scalar.activation(ex[:], lg[:, :K], AF.Exp)
            ssum = smp.tile([P, 1], F32, tag="ssum")
            nc.vector.reduce_sum(ssum[:], ex[:], axis=mybir.AxisListType.X)
            rec = smp.tile([P, 1], F32, tag="rec")
            nc.vector.reciprocal(rec[:], ssum[:])
            nc.vector.tensor_scalar_mul(alpha[:, t * K:(t + 1) * K], ex[:], rec[:])

            if any_bias:
                at_ps = zp.tile([P, D], F32, tag="z")
                nc.tensor.transpose(
                    at_ps[:K, :P], alpha[:, t * K:(t + 1) * K], ident_sb[:]
                )
                nc.scalar.activation(
                    alphaT[:, t * P:(t + 1) * P], at_ps[:K, :P], AF.Copy
                )

        # ---- 4 MoIE blocks (x repeat for timing builds) ----
        # two-stage deferral: the relu for tile t is emitted at tile t+1
        # (an engine-queue op; never blocks the ACT sequencer), and the
        # X-bar transpose at tile t+2 -- by then its input has long been
        # materialized, so the transpose's wait no longer parks on the
        # ACT SEQ (which a DMA instruction, unlike a compute op, holds
        # until its inputs are ready).
        pend_relu = []
        pend_tr = []

        def advance():
            if pend_tr:
                src_t, dst = pend_tr.pop(0)
                ht_n = htp.tile([P, NCH, P], MD, tag="ht")
                nc.sync.dma_start_transpose(ht_n[:], src_t[:])
                hT[dst] = ht_n
            if pend_relu:
                accb, dst, pli = pend_relu.pop(0)
                if pli < 3:
                    accr = accbp.tile([P, D], MD, tag="accr")
                    nc.scalar.activation(accr[:], accb[:], AF.Relu)
                    pend_tr.append((accr, dst))
                else:
                    pend_tr.append((accb, dst))

        for gli in range(4 * repeat):
            li = gli % 4
            last = gli == 4 * repeat - 1
            if not last:
                nli = (gli + 1) % 4
                wt_next = [wpool.tile([P, NCH, D], MD, tag="w",
                                      name=f"w_{gli + 1}_{k}")
                           for k in range(K)]

            for t in range(NT):
                # next layer's weight DMAs, one per tile: keeps the DMA
                # queues clear of 8-transfer bursts at layer boundaries
                if not last and t < K:
                    nc.scalar.dma_start(wt_next[t][:], wsrs[nli][t])
                bias_sb = None
                if has_bias[li]:
                    b_ps = zp.tile([P, D], F32, tag="z")
                    nc.tensor.matmul(
                        b_ps[:],
                        alphaT[:, t * P:(t + 1) * P],
                        bl_sb[li][:],
                    )
                    bias_sb = smp.tile([P, D], F32, tag="bias_sb")
                    nc.scalar.activation(bias_sb[:], b_ps[:], AF.Copy)

                while (gli, t) not in hT:
                    advance()
                ht_in = hT[(gli, t)]
                zs = []
                for k in range(K):
                    z = zp.tile([P, D], F32, tag="z", name=f"z_{gli}_{t}_{k}")
                    for c in range(NCH):
                        nc.tensor.matmul(
                            z[:],
                            ht_in[:, c, :],
                            wt[k][:, c, :],
                            start=(c == 0),
                            stop=(c == NCH - 1),
                        )
                    zs.append(z)

                if mode == "pe_only":
                    continue

                # weighted expert sum: sk1/sk3 on ScalarE, gs = sk1+sk3 on
                # VectorE seeds the STT chain (no standalone final add); the
                # last STT writes the bf16 combine result directly.
                def a_ap(k):
                    return alpha[:, t * K + k:t * K + k + 1]

                sks = {}
                for k in (1, 3, 5, 7):
                    sk = smp.tile([P, D], F32, tag="sk", bufs=8)
                    nc.scalar.activation(sk[:], zs[k][:], AF.Copy,
                                         scale=a_ap(k))
                    sks[k] = sk

                acc = accp.tile([P, D], F32, tag="acc")
                if bias_sb is not None:
                    nc.vector.scalar_tensor_tensor(
                        acc[:], zs[0][:], a_ap(0), bias_sb[:],
                        op0=ALU.mult, op1=ALU.add,
                    )
                else:
                    nc.vector.tensor_scalar_mul(acc[:], zs[0][:], a_ap(0))
                nc.vector.scalar_tensor_tensor(
                    acc[:], zs[2][:], a_ap(2), acc[:],
                    op0=ALU.mult, op1=ALU.add,
                )
                g1 = smp.tile([P, D], F32, tag="gs", bufs=4)
                nc.vector.tensor_add(g1[:], sks[1][:], sks[3][:])
                nc.vector.scalar_tensor_tensor(
                    acc[:], zs[4][:], a_ap(4), acc[:],
                    op0=ALU.mult, op1=ALU.add,
                )
                nc.vector.scalar_tensor_tensor(
                    acc[:], zs[6][:], a_ap(6), acc[:],
                    op0=ALU.mult, op1=ALU.add,
                )
                # staged relu/transpose of previous tiles ride ACT behind
                # this tile's sk reads (see advance() above)
                advance()
                nc.vector.tensor_add(acc[:], acc[:], g1[:])
                g2 = smp.tile([P, D], F32, tag="gs", bufs=4)
                nc.vector.tensor_add(g2[:], sks[5][:], sks[7][:])
                if last:
                    outt = accp.tile([P, D], F32, tag="acc")
                    nc.vector.tensor_add(outt[:], acc[:], g2[:])
                    nc.sync.dma_start(out[t * P:(t + 1) * P, :], outt[:])
                else:
                    accb = accbp.tile([P, D], MD, tag="accb")
                    nc.vector.tensor_add(accb[:], acc[:], g2[:])
                    pend_relu.append((accb, (gli + 1, t), li))

            if not last:
                wt = wt_next

        while pend_relu or pend_tr:
            advance()

    nc.compile()
    return nc


_CACHE = {}


def _get_nc(key):
    if key not in _CACHE:
        _CACHE[key] = _build(key[0], key[1])
    return _CACHE[key]


def _to_md(a):
    import ml_dtypes

    return np.ascontiguousarray(np.asarray(a, dtype=np.float32).astype(
        ml_dtypes.bfloat16))


def kernel(**inputs):
    x = _to_md(inputs["x"])
    gate_w = _to_md(inputs["gate_w"])
    gate_b = np.ascontiguousarray(np.asarray(inputs["gate_b"], dtype=np.float32))
    wlist = [_to_md(inputs[n]) for n in W_NAMES]
    blist = [np.ascontiguousarray(np.asarray(inputs[n], dtype=np.float32))
             for n in B_NAMES]

    has_gate_b = bool(np.any(gate_b))
    has_bias = tuple(bool(np.any(b)) for b in blist)
    nc = _get_nc((has_gate_b, has_bias))

    shared = {"gate_w": gate_w, "gate_b": gate_b}
    for n, w in zip(W_NAMES, wlist):
        shared[n] = w
    for n, b in zip(B_NAMES, blist):
        shared[n] = b
    if any(has_bias):
        shared["ident"] = np.eye(P, dtype=np.float32)

    core_ids = list(range(NCORES))
    in_maps = [dict(shared, x=x[i * R:(i + 1) * R]) for i in core_ids]
    res = run_bass_kernel_spmd(nc, in_maps, core_ids)
    return np.concatenate([res.results[i]["out"] for i in core_ids], axis=0)


if __name__ == "__main__":
    rng = np.random.default_rng(0)
    ins = {
        "x": rng.standard_normal((B, D), dtype=np.float32),
        "gate_w": rng.standard_normal((D, K), dtype=np.float32) * 0.02,
        "gate_b": np.zeros((K,), np.float32),
    }
    for n in W_NAMES:
        ins[n] = rng.standard_normal((K, D, D), dtype=np.float32) * 0.02
    for n in B_NAMES:
        ins[n] = np.zeros((K, D), np.float32)
    y = kernel(**ins)
    print("out", y.shape, y.dtype, float(np.abs(y).max()))
